# revision 2
# baseline (speedup 1.0000x reference)
"""Trainium2 Bass kernel for nn_CrossAttention (B=2, Lq=Lkv=2048, E=1024, H=16, D=64).

Tensor-parallel over heads: each of 8 cores owns 2 heads (JC=128 features
of QKV, matching 128 columns of Wo); host sums the 8 output partials.

v2 design (vs baseline):
  - bf16 activations/weights everywhere on the DMA path (halves HBM traffic;
    PE rate for bf16 == fp32r at 1 cyc/row, so no compute cost).
  - V is produced directly in [kv_token, feature] orientation by using the
    x^T input tiles as the matmul *stationary* operand (contraction over E),
    eliminating the baseline's PE-transpose + 4-DVE-copies phase entirely.
    V bias is folded in via a rank-1 matmul (ones x bv row).
  - Context matmuls contract the full 128 kv positions per chunk (the
    baseline split them into 2x64 and re-added, doubling PE time).
  - A ones column rides as column 0/65 of the V stationary so each context
    matmul also accumulates the softmax denominator in PSUM rows 0/65.
  - Normalized context is stored per head in separate 65-partition SBUF
    tensors (ctxA/ctxB); the Wo projection contracts each with a 65-row
    Wo half (row 0 zeroed), so no cross-partition DMA shuffle is needed.
  - Scores are emitted one kt ahead of context matmuls so the PE never
    waits on the ScalarE exp (ScalarE paces the attention inner loop).
  - Output written as bf16 partials, one DMA per 1024-token tile.
"""

import sys

if "/opt/trn_rl_repo" not in sys.path:
    sys.path.insert(0, "/opt/trn_rl_repo")

import numpy as np

import concourse.tile as tile
from concourse import bacc, mybir
from concourse.bass_utils import run_bass_kernel_spmd

F32 = mybir.dt.float32
F32R = mybir.dt.float32r
BF16 = mybir.dt.bfloat16
AF = mybir.ActivationFunctionType
NPBF16 = mybir.dt.np(BF16)

N_CORES = 8
B, LQ, LKV, E, H, D = 2, 2048, 2048, 1024, 16, 64
HC = H // N_CORES  # heads per core = 2
JC = HC * D  # feature slice per core = 128
T = B * LQ  # 4096 tokens
NEC = E // 128  # 8 e-chunks
NTP = T // 1024  # 4 big token tiles (projections / output)
NQT = LQ // 512  # 4 q tiles per batch (attention)
NKT = LKV // 128  # 16 k chunks per batch
GC = B * NKT  # 32 kv chunks total
NOC = E // 128  # 8 output chunks

_NC_CACHE = {}


def build(reps=None, phases="PAO"):
    key = (reps or 0, phases)
    if key in _NC_CACHE:
        return _NC_CACHE[key]
    nc = bacc.Bacc("TRN2", target_bir_lowering=False, debug=False, num_devices=N_CORES)

    xqT = nc.dram_tensor("xqT", [E, T], BF16, kind="ExternalInput").ap()
    xkT = nc.dram_tensor("xkT", [E, T], BF16, kind="ExternalInput").ap()
    wqT = nc.dram_tensor("wqT", [E, JC], BF16, kind="ExternalInput").ap()
    wkT = nc.dram_tensor("wkT", [E, JC], BF16, kind="ExternalInput").ap()
    wvT = nc.dram_tensor("wvT", [E, JC], BF16, kind="ExternalInput").ap()
    woA = nc.dram_tensor("woA", [65, E], BF16, kind="ExternalInput").ap()
    woB = nc.dram_tensor("woB", [65, E], BF16, kind="ExternalInput").ap()
    bqd = nc.dram_tensor("bq", [JC, 1], F32, kind="ExternalInput").ap()
    bkd = nc.dram_tensor("bk", [JC, 1], F32, kind="ExternalInput").ap()
    bvd = nc.dram_tensor("bv", [1, JC], BF16, kind="ExternalInput").ap()
    bod = nc.dram_tensor("bo", [NOC, 128], F32, kind="ExternalInput").ap()
    mbd = nc.dram_tensor("mb", [B, NKT, 128], F32, kind="ExternalInput").ap()
    outT = nc.dram_tensor("outT", [E, T], BF16, kind="ExternalOutput").ap()

    from contextlib import nullcontext

    with tile.TileContext(nc) as tc, nc.allow_low_precision(reason="bf16 kernel"):
        with tc.For_i(0, reps, 1) if reps else nullcontext():
         with (
             tc.tile_pool(name="const", bufs=1) as const,
             tc.tile_pool(name="big", bufs=1) as big,
         ):
             # ---- persistent SBUF state ----
             wq_sb = const.tile([128, NEC, JC], BF16, tag="wq")
             nc.sync.dma_start(out=wq_sb, in_=wqT.rearrange("(ec p) j -> p ec j", p=128))
             wk_sb = const.tile([128, NEC, JC], BF16, tag="wk")
             nc.sync.dma_start(out=wk_sb, in_=wkT.rearrange("(ec p) j -> p ec j", p=128))
             wv_sb = const.tile([128, NEC, JC], BF16, tag="wv")
             nc.sync.dma_start(out=wv_sb, in_=wvT.rearrange("(ec p) j -> p ec j", p=128))
             woA_sb = const.tile([65, NOC, 128], BF16, tag="woA")
             nc.sync.dma_start(out=woA_sb, in_=woA.rearrange("p (oc o) -> p oc o", oc=NOC))
             woB_sb = const.tile([65, NOC, 128], BF16, tag="woB")
             nc.sync.dma_start(out=woB_sb, in_=woB.rearrange("p (oc o) -> p oc o", oc=NOC))
             bq_sb = const.tile([128, 1], F32, tag="bq")
             nc.sync.dma_start(out=bq_sb, in_=bqd)
             bk_sb = const.tile([128, 1], F32, tag="bk")
             nc.sync.dma_start(out=bk_sb, in_=bkd)
             bv_sb = const.tile([1, JC], BF16, tag="bv")
             nc.sync.dma_start(out=bv_sb, in_=bvd)
             bo_sb = const.tile([128, NOC], F32, tag="bo")
             nc.sync.dma_start(out=bo_sb, in_=bod.rearrange("oc o -> o oc"))
             mb_sb = const.tile([128, B, NKT], F32, tag="mb")
             nc.sync.dma_start(out=mb_sb, in_=mbd.rearrange("b kc p -> p b kc"))
             ones_f = const.tile([1, 128], F32, tag="onesf")
             nc.vector.memset(ones_f, 1.0)
             ones_b = const.tile([1, 128], BF16, tag="onesb")
             nc.vector.tensor_copy(ones_b, ones_f)
             onesc = const.tile([1, 65], F32R, tag="onesc")
             nc.vector.tensor_copy(onesc, ones_f[:, 0:65])

             qt_sb = big.tile([128, T], BF16, tag="qt")
             kt_sb = big.tile([128, T], BF16, tag="kt")
             v_sb = big.tile([128, GC, 130], BF16, tag="v")
             ctxA = big.tile([65, NTP, 1024], BF16, tag="ctxA")
             ctxB = big.tile([65, NTP, 1024], BF16, tag="ctxB")

             # ones columns (denominator trick) for all kv chunks
             nc.vector.memset(v_sb[:, :, 0:1], 1.0)
             nc.vector.memset(v_sb[:, :, 65:66], 1.0)

             # ---- phase P: projections ----
             if "P" in phases:
              with (
                 tc.tile_pool(name="xin", bufs=2) as xin,
                 tc.tile_pool(name="pp", bufs=2, space="PSUM") as pp,
                 tc.tile_pool(name="vp", bufs=3, space="PSUM") as vp,
             ):
                 # kv tiles first (attention needs all K/V of a batch), then q
                 for isq, xsrc, wsb, bias, dst in (
                     (False, xkT, wk_sb, bk_sb, kt_sb),
                     (True, xqT, wq_sb, bq_sb, qt_sb),
                 ):
                     for tp in range(NTP):
                         t0 = tp * 1024
                         xt = xin.tile([128, NEC, 1024], BF16, tag="xin")
                         nc.sync.dma_start(
                             out=xt,
                             in_=xsrc[:, t0 : t0 + 1024].rearrange(
                                 "(ec p) t -> p ec t", p=128
                             ),
                         )
                         pt = pp.tile([128, 2, 512], F32, tag="pp")
                         for h in range(2):
                             for ec in range(NEC):
                                 nc.tensor.matmul(
                                     pt[:, h, :],
                                     wsb[:, ec, :],
                                     xt[:, ec, h * 512 : (h + 1) * 512],
                                     start=(ec == 0),
                                     stop=(ec == NEC - 1),
                                 )
                         nc.vector.tensor_scalar_add(
                             dst[:, t0 : t0 + 1024],
                             pt.rearrange("p a t -> p (a t)"),
                             bias,
                         )
                         if not isq:
                             # V for the 8 kv chunks of this tile: x^T slice as
                             # stationary -> V in [kv, feat] orientation
                             for c in range(8):
                                 gc = tp * 8 + c
                                 vt = vp.tile([128, 128], F32, tag="vp")
                                 for ec in range(NEC):
                                     nc.tensor.matmul(
                                         vt,
                                         xt[:, ec, c * 128 : (c + 1) * 128],
                                         wv_sb[:, ec, :],
                                         start=(ec == 0),
                                         stop=False,
                                     )
                                 nc.tensor.matmul(
                                     vt, ones_b, bv_sb, start=False, stop=True
                                 )
                                 nc.vector.tensor_copy(
                                     v_sb[:, gc, :].rearrange(
                                         "p (a c) -> p a c", a=2
                                     )[:, :, 1:65],
                                     vt.rearrange("p (a c) -> p a c", a=2),
                                 )

             # ---- phase A: attention ----
             if "A" in phases:
              with (
                 tc.tile_pool(name="attps", bufs=2, space="PSUM") as attps,
                 tc.tile_pool(name="cxps", bufs=2, space="PSUM") as cxps,
                 tc.tile_pool(name="expm", bufs=3) as expm,
                 tc.tile_pool(name="dv", bufs=2) as dv,
             ):
                 for b in range(B):
                     for qt in range(NQT):
                         q0 = b * LQ + qt * 512
                         cxt = cxps.tile([65, 2, 512], F32, tag="cx")
                         scts = {}

                         def scores(kt):
                             k0 = b * LKV + kt * 128
                             sct = attps.tile([128, 2, 512], F32, tag="sc")
                             nc.tensor.matmul(
                                 sct[:, 0, :],
                                 kt_sb[0:64, k0 : k0 + 128],
                                 qt_sb[0:64, q0 : q0 + 512],
                                 start=True, stop=True,
                             )
                             nc.tensor.matmul(
                                 sct[:, 1, :],
                                 kt_sb[64:128, k0 : k0 + 128],
                                 qt_sb[64:128, q0 : q0 + 512],
                                 start=True, stop=True,
                             )
                             scts[kt] = sct

                         scores(0)
                         for kt in range(NKT):
                             if kt + 1 < NKT:
                                 scores(kt + 1)
                             sct = scts.pop(kt)
                             emt = expm.tile([128, 2, 512], BF16, tag="expm")
                             nc.scalar.activation(
                                 out=emt.rearrange("p a t -> p (a t)"),
                                 in_=sct.rearrange("p a t -> p (a t)"),
                                 func=AF.Exp,
                                 bias=mb_sb[:, b, kt : kt + 1],
                                 scale=0.125,
                             )
                             st, sp = (kt == 0), (kt == NKT - 1)
                             gc = b * NKT + kt
                             nc.tensor.matmul(
                                 cxt[:, 0, :], v_sb[:, gc, 0:65], emt[:, 0, :],
                                 start=st, stop=sp,
                             )
                             nc.tensor.matmul(
                                 cxt[:, 1, :], v_sb[:, gc, 65:130], emt[:, 1, :],
                                 start=st, stop=sp,
                             )

                         # normalize: ctx_h = raw_ctx_h * (1/denom_h)
                         tt = b * NQT + qt  # 512-token tile index
                         tp, half = tt // 2, (tt % 2) * 512
                         s2r = dv.tile([65, 2, 512], F32, tag="s2r")
                         nc.vector.tensor_copy(s2r, cxt)
                         rr = dv.tile([1, 2, 512], F32R, tag="rr")
                         nc.vector.reciprocal(
                             rr.rearrange("p a t -> p (a t)"),
                             s2r[0:1, :, :].rearrange("p a t -> p (a t)"),
                         )
                         bct = cxps.tile([65, 2, 512], F32, tag="cx")
                         nc.tensor.matmul(
                             bct[:, 0, :], onesc, rr[:, 0, :], start=True, stop=True
                         )
                         nc.tensor.matmul(
                             bct[:, 1, :], onesc, rr[:, 1, :], start=True, stop=True
                         )
                         nc.vector.tensor_mul(
                             ctxA[:, tp, half : half + 512], s2r[:, 0, :], bct[:, 0, :]
                         )
                         nc.vector.tensor_mul(
                             ctxB[:, tp, half : half + 512], s2r[:, 1, :], bct[:, 1, :]
                         )

             # ---- phase O: output projection (partial; host sums cores) ----
             if "O" in phases:
              with (
                 tc.tile_pool(name="ops", bufs=3, space="PSUM") as ops,
                 tc.tile_pool(name="outsb", bufs=2) as outsb,
             ):
                 for tp in range(NTP):
                     ob = outsb.tile([128, NOC, 1024], BF16, tag="ob")
                     for oc in range(NOC):
                         opt = ops.tile([128, 2, 512], F32, tag="op")
                         for h in range(2):
                             sl = slice(h * 512, (h + 1) * 512)
                             nc.tensor.matmul(
                                 opt[:, h, :], woA_sb[:, oc, :], ctxA[:, tp, sl],
                                 start=True, stop=False,
                             )
                             nc.tensor.matmul(
                                 opt[:, h, :], woB_sb[:, oc, :], ctxB[:, tp, sl],
                                 start=False, stop=True,
                             )
                         if oc % 2 == 0:
                             nc.vector.tensor_scalar_add(
                                 ob[:, oc, :],
                                 opt.rearrange("p a t -> p (a t)"),
                                 bo_sb[:, oc : oc + 1],
                             )
                         else:
                             nc.scalar.activation(
                                 out=ob[:, oc, :],
                                 in_=opt.rearrange("p a t -> p (a t)"),
                                 func=AF.Identity,
                                 bias=bo_sb[:, oc : oc + 1], scale=1.0,
                             )
                     nc.sync.dma_start(
                         out=outT[:, tp * 1024 : (tp + 1) * 1024].rearrange(
                             "(oc p) t -> p oc t", p=128
                         ),
                         in_=ob,
                     )

    nc.compile()
    _NC_CACHE[key] = nc
    return nc


def _bf16(a):
    return np.ascontiguousarray(a).astype(NPBF16)


def make_in_maps(query, key_value, mask, Wq, bq, Wk, bk, Wv, bv, Wo, bo):
    xqT = _bf16(query.reshape(T, E).T)
    xkT = _bf16(key_value.reshape(T, E).T)
    mb = np.where(mask != 0, 0.0, -1.0e5).astype(np.float32).reshape(B, NKT, 128)
    in_maps = []
    for c in range(N_CORES):
        sl = slice(c * JC, (c + 1) * JC)
        woA = np.zeros((65, E), np.float32)
        woA[1:, :] = Wo[:, c * JC : c * JC + 64].T
        woB = np.zeros((65, E), np.float32)
        woB[1:, :] = Wo[:, c * JC + 64 : (c + 1) * JC].T
        in_maps.append(
            {
                "xqT": xqT,
                "xkT": xkT,
                "wqT": _bf16(Wq[sl, :].T),
                "wkT": _bf16(Wk[sl, :].T),
                "wvT": _bf16(Wv[sl, :].T),
                "woA": _bf16(woA),
                "woB": _bf16(woB),
                "bq": bq[sl].reshape(JC, 1).astype(np.float32),
                "bk": bk[sl].reshape(JC, 1).astype(np.float32),
                "bv": _bf16(bv[sl].reshape(1, JC)),
                # only core 0 adds bo so the host-side partial sum sees it once
                "bo": (
                    bo.reshape(NOC, 128).astype(np.float32)
                    if c == 0
                    else np.zeros((NOC, 128), np.float32)
                ),
                "mb": mb,
            }
        )
    return in_maps


def kernel(query, key_value, mask, Wq, bq, Wk, bk, Wv, bv, Wo, bo):
    nc = build()
    in_maps = make_in_maps(
        np.asarray(query), np.asarray(key_value), np.asarray(mask),
        np.asarray(Wq), np.asarray(bq), np.asarray(Wk), np.asarray(bk),
        np.asarray(Wv), np.asarray(bv), np.asarray(Wo), np.asarray(bo),
    )
    res = run_bass_kernel_spmd(nc, in_maps, list(range(N_CORES)))
    acc = np.zeros((E, T), np.float32)
    for c in range(N_CORES):
        acc += res.results[c]["outT"].astype(np.float32)
    return np.ascontiguousarray(acc.T).reshape(B, LQ, E).astype(np.float32)
